# revision 52
# baseline (speedup 1.0000x reference)
"""MoE layer (dense experts) on 8 Trainium2 NeuronCores via Bass/Tile.

Problem (hardcoded shapes):
  x        [4, 2048, 1024] f32
  gate_w   [1024, 8] f32, gate_b [8] f32
  expert_w [8, 1024, 1024] f32, expert_b [8, 1024] f32
  out[b,t,p] = sum_e softmax(x @ gate_w + gate_b)[b,t,e]
               * (x @ expert_w[e] + expert_b[e])[b,t,p]

Sharding: data-parallel over tokens. 8192 tokens are split into 8 shards of
1024; every core gets the full gate/expert weights (replicated) and computes
its token shard end-to-end. No collectives.

The kernel is TensorEngine-streaming-bound: 8 experts x 8 token tiles x
2 p-chunks x 8 d-tiles = 1024 bf16 matmuls of [128]x[128,512] at ~214ns
each (512 cycles @2.4GHz) ~= 219us/core floor. fp8 DoubleRow was measured
on HW at the same per-instruction duration as bf16 (2x MACs/instr, i.e.
157 TF/s); an x1w1+x1w2+x2w1 split-fp8 scheme costs 1.5x bf16 cycles, so
bf16 is optimal for the 2e-2 error budget. The rest of the design tightens
the ramp/tail around the matmul stream (265us -> 244us measured):
  - PE warmup matmuls on memset SBUF so the p-state clock ramp
    (0.65->2.4GHz) elapses during the initial DMA wait.
  - xi/w0 DMAs interleaved per d-tile; expert-1's first d-tiles prefetch
    inside the ramp; expert-0 runs d-outer on 6 banks while gating
    accumulates into ONE shared psum bank ([128, 8] regions, single
    bank-granular start via the warm tile or the gate_b rank-1 matmul).
  - softmax emits only DVE/ACT ops; waveB's first group borrows the aux
    bank so the in-order PE never waits on the softmax semaphore chain.
  - epilogue is one DVE scalar_tensor_tensor per (expert, tile); the
    final expert's STT leaves the tile in acc, DMA'd straight out (the
    last tile split across two queues); the gate-transpose + gate-weighted
    expert_b mix work is deferred into experts 1-2 (and compiled out
    entirely when both biases are zero, which the spec guarantees).
Matmul dtype: bf16 (default) or float32r via MOE_MM_DTYPE in
{bf16, fp32r, fp32}.
"""

import os
from contextlib import ExitStack

import numpy as np

import concourse.bacc as bacc
import concourse.bass as bass
import concourse.mybir as mybir
import concourse.tile as tile
from concourse.bass_utils import run_bass_kernel_spmd

B, T, D, E, P = 4, 2048, 1024, 8, 1024
N_CORES = 8
TOK = B * T                # 8192 tokens
TS = TOK // N_CORES        # 1024 tokens per core
DT = D // 128              # 8 contraction tiles
TT = TS // 128             # 8 token tiles per core
PCHUNK = 512               # psum bank free size (f32)
PC = P // PCHUNK           # 2 p-chunks
WAVE_A = 3                 # token tiles computed d-outer during the DMA ramp

_F32 = mybir.dt.float32
_BF16 = mybir.dt.bfloat16

MM_DTYPE = os.environ.get("MOE_MM_DTYPE", "bf16")
TRACE = os.environ.get("MOE_TRACE", "0") == "1"  # test.py sets this

_mm_dt = {
    "fp32r": mybir.dt.float32r,
    "bf16": mybir.dt.bfloat16,
    "fp32": mybir.dt.float32,
}

_build_cache = {}


def _build(mode: str, no_bias: bool = False) -> bass.Bass:
    """no_bias=True compiles out the gate_b add and the gate-weighted
    expert_b mix (both inputs are spec'd fill=zeros; kernel() checks the
    actual arrays at call time and falls back to the general build)."""
    mm = _mm_dt[mode]
    nc = bacc.Bacc("TRN2", target_bir_lowering=False, debug=False,
                   num_devices=N_CORES)

    xT = nc.dram_tensor("xT", [D, TS], mm, kind="ExternalInput").ap()
    gw = nc.dram_tensor("gate_w", [D, E], mm, kind="ExternalInput").ap()
    ew = nc.dram_tensor("expert_w", [E, D, P], mm, kind="ExternalInput").ap()
    if not no_bias:
        gbr = nc.dram_tensor("gb_rep", [1, TT * E], mm,
                             kind="ExternalInput").ap()
        eb = nc.dram_tensor("expert_b", [E, P], _BF16,
                            kind="ExternalInput").ap()
        ones = nc.dram_tensor("ones", [1, 128], mm, kind="ExternalInput").ap()
        ident = nc.dram_tensor("ident", [128, 128], _F32,
                               kind="ExternalInput").ap()
    out = nc.dram_tensor("out", [TS, P], _F32, kind="ExternalOutput").ap()

    out_t = out.rearrange("(tt tp) p -> tp tt p", tp=128)
    xT_t = xT.rearrange("(dt dp) t -> dp dt t", dp=128)

    with tile.TileContext(nc) as tc, ExitStack() as ctx:
        consts = ctx.enter_context(tc.tile_pool(name="consts", bufs=1))
        w_pool = ctx.enter_context(tc.tile_pool(name="w", bufs=28))
        stats = ctx.enter_context(tc.tile_pool(name="stats", bufs=4))
        # main pool: matmul accumulation banks (+ the gating bank);
        # aux pool: one bank serializing gate-transpose / expert_b-mix
        # scratch, kept separate so its PE->ACT->PE recycle chain can never
        # interleave with matmul-group bank recycling.
        psum = ctx.enter_context(tc.tile_pool(name="psum", bufs=7, space="PSUM"))
        psum_aux = ctx.enter_context(tc.tile_pool(name="psx", bufs=1, space="PSUM"))

        # PE warmup: dummy matmuls on memset SBUF with no DMA dependency, so
        # the tensor engine's p-state ramp (0.65->2.4GHz over ~3us of
        # activity) elapses while the first input DMAs are still in flight
        # instead of while real matmuls run.
        n_warm = int(os.environ.get("MOE_WARMUP", "16"))
        warm = consts.tile([128, PCHUNK], mm, name="warm")
        nc.any.memset(warm[:, :], 0)
        if n_warm:
            wps = psum_aux.tile([128, PCHUNK], _F32, name="wps", tag="px")
            for _ in range(n_warm):
                nc.tensor.matmul(wps[:, :], warm[:, :128], warm[:, :],
                                 start=True, stop=True)

        # Tiny resident inputs first so the gating-bias matmul can issue
        # immediately; xi and expert-0 weights interleave per d-tile so the
        # expert-0 d-outer pipeline starts as soon as the first pair lands.
        gw_sb = consts.tile([128, DT, E], mm, name="gw_sb")
        nc.sync.dma_start(gw_sb[:, :, :], gw.rearrange("(dt dp) e -> dp dt e", dp=128))
        if not no_bias:
            ones_sb = consts.tile([1, 128], mm, name="ones_sb")
            nc.sync.dma_start(ones_sb[:, :], ones)
            gbr_sb = consts.tile([1, TT * E], mm, name="gbr_sb")
            nc.sync.dma_start(gbr_sb[:, :], gbr)
            eb_sb = consts.tile([E, P], _BF16, name="eb_sb")
            nc.sync.dma_start(eb_sb[:, :], eb)
            id_sb = consts.tile([128, 128], _F32, name="id_sb")
            nc.sync.dma_start(id_sb[:, :], ident)

        def w_dma(e, di):
            w_tile = w_pool.tile([128, P], mm, name=f"wt{e}_{di}", tag="wt")
            nc.sync.dma_start(w_tile[:, :], ew[e, di * 128:(di + 1) * 128, :])
            return w_tile

        xt = consts.tile([128, DT, TS], mm, name="xt")
        w0 = []
        w1_pre = []
        for di in range(DT):
            nc.sync.dma_start(xt[:, di, :], xT_t[:, di, :])
            w0.append(w_dma(0, di))
            if di >= 6:
                # prefetch expert-1's first d-tiles into the ramp so e1's
                # first groups don't wait on a cold DMA stream
                w1_pre.append(w_dma(1, di - 6))

        g_sb = consts.tile([128, TT, E], _F32, name="g_sb")
        if not no_bias:
            gt_sb = consts.tile([E, TS], _BF16, name="gt_sb")
        acc = consts.tile([128, TT, P], _F32, name="acc")

        # One psum bank holds every token tile's gating logits as [128, E]
        # regions, so gating+softmax occupies a single bank and expert-0 can
        # run d-outer on 6 banks concurrently.
        # PSUM start_tensor_calc is bank-granular (2KB zero region), so there
        # must be exactly ONE starting matmul for the shared gating bank; all
        # per-token-tile logit matmuls accumulate with start=False and rely
        # on zero-fill-on-first-touch. With biases, the starter also adds
        # gate_b (ones.T @ gb_rep); without, it writes zeros from warm.
        # Allocated AFTER the waveA banks (below) so waveB's groups rotate
        # onto waveA's slots (freed by the first epilogue STTs) rather than
        # this bank's slot, which frees only after the whole softmax chain
        # has read the logits. Pool slots rotate by allocation index.
        ps_grp = {}
        for ti in range(WAVE_A):
            for pc in range(PC):
                ps_grp[ti, pc] = psum.tile([128, PCHUNK], _F32,
                                           name=f"ps0_{ti}_{pc}", tag="ps")
        gbank = psum.tile([128, PCHUNK], _F32, name="gbank", tag="ps")
        lg_all = gbank[:, 0:TT * E]
        if no_bias:
            nc.tensor.matmul(lg_all, warm[:1, :128], warm[:1, :TT * E],
                             start=True, stop=False, skip_group_check=True)
        else:
            nc.tensor.matmul(lg_all, ones_sb[:1, :], gbr_sb[:1, :],
                             start=True, stop=False, skip_group_check=True)

        def gate_mms(di):
            for ti in range(TT):
                nc.tensor.matmul(gbank[:, ti * E:(ti + 1) * E],
                                 xt[:, di, ti * 128:(ti + 1) * 128],
                                 gw_sb[:, di, :],
                                 start=False, stop=(di == DT - 1),
                                 skip_group_check=True)

        # --- expert-0 wave A: d-outer over 6 banks, gating interleaved ---
        for di in range(DT):
            gate_mms(di)
            for ti in range(WAVE_A):
                for pc in range(PC):
                    nc.tensor.matmul(
                        ps_grp[ti, pc][:, :],
                        xt[:, di, ti * 128:(ti + 1) * 128],
                        w0[di][:, pc * PCHUNK:(pc + 1) * PCHUNK],
                        start=(di == 0), stop=(di == DT - 1))

        # --- softmax: DVE/ACT only, no PE instructions, so waveB's matmuls
        # stream on the in-order PE immediately after waveA's. The gate
        # transposes (PE) are deferred and interleaved into expert 1's
        # groups below.
        for ti in range(TT):
            lg = gbank[:, ti * E:(ti + 1) * E]
            negmax = stats.tile([128, 1], _F32, name="negmax")
            nc.vector.tensor_reduce(negmax[:, :], lg, axis=mybir.AxisListType.X,
                                    op=mybir.AluOpType.max, negate=True)
            gexp = g_sb[:, ti, :]
            esum = stats.tile([128, 1], _F32, name="esum")
            nc.scalar.activation(gexp, lg, mybir.ActivationFunctionType.Exp,
                                 bias=negmax[:, :], scale=1.0,
                                 accum_out=esum[:, :])
            rec = stats.tile([128, 1], _F32, name="rec")
            nc.vector.reciprocal(rec[:, :], esum[:, :])
            nc.vector.tensor_scalar_mul(gexp, gexp, rec[:, :])

        # --- epilogues ---
        # e == 0 initializes acc = ps*g with no dependency on the gate
        # transposes or expert_b mix, so psum banks recycle as soon as each
        # token tile's own softmax lands. The gate-weighted expert_b mix is
        # floated into the long window between expert 0 and expert 7 (see
        # below) rather than sitting on either the ramp or the tail.
        def epilogue(e, ti, pc, ps):
            g_col = g_sb[:, ti, e:e + 1]
            acc_sl = acc[:, ti, pc * PCHUNK:(pc + 1) * PCHUNK]
            if e == 0:
                nc.vector.tensor_scalar_mul(acc_sl, ps[:, :], g_col)
            else:
                nc.vector.scalar_tensor_tensor(
                    acc_sl, ps[:, :], g_col, acc_sl,
                    op0=mybir.AluOpType.mult, op1=mybir.AluOpType.add)
            if e == E - 1:
                sl = slice(pc * PCHUNK, (pc + 1) * PCHUNK)
                if ti == TT - 1:
                    # The final tile's stores are exposed on the kernel tail
                    # and a single queue only moves ~33GB/s: split by
                    # partition (keeps the 2KB line) across two queues.
                    nc.sync.dma_start(out_t[:64, ti, sl], acc[:64, ti, sl])
                    nc.sync.dma_start(out_t[64:, ti, sl], acc[64:, ti, sl])
                else:
                    nc.sync.dma_start(out_t[:, ti, sl], acc_sl)

        for ti in range(WAVE_A):
            for pc in range(PC):
                epilogue(0, ti, pc, ps_grp[ti, pc])

        def expert_group(e, wt, ti, pc, pool=None):
            ps = (pool or psum).tile([128, PCHUNK], _F32,
                                     name=f"ps{e}_{ti}_{pc}",
                                     tag="px" if pool else "ps")
            for di in range(DT):
                nc.tensor.matmul(
                    ps[:, :], xt[:, di, ti * 128:(ti + 1) * 128],
                    wt[di][:, pc * PCHUNK:(pc + 1) * PCHUNK],
                    start=(di == 0), stop=(di == DT - 1))
            epilogue(e, ti, pc, ps)

        # expert-0 wave B: remaining token tiles, group-major. The first
        # group borrows the aux bank (idle since warmup) so the in-order PE
        # streams straight through the waveA->waveB boundary while the
        # serial softmax/semaphore chain releases the main banks.
        for ti in range(WAVE_A, TT):
            for pc in range(PC):
                expert_group(0, w0, ti, pc,
                             pool=psum_aux if (ti, pc) == (WAVE_A, 0)
                             else None)

        # Deferred small PE work, one op per expert group so the serialized
        # aux-bank chains (transpose -> ACT copy, bias matmul -> DVE add)
        # always have a full 8-matmul group executing between them:
        #   - 8 gate transposes (for the expert_b mix) during expert 1
        #   - 16 gate-weighted expert_b mix tiles (acc += gT.T @ expert_b)
        #     during experts 1-2; each lands after its tile's e==0 init and
        #     long before its e==7 STT.
        deferred = []
        if not no_bias:
            for ti in range(TT):
                def mk_t(ti=ti):
                    ps_t = psum_aux.tile([128, PCHUNK], _F32, name="ps_t",
                                         tag="px")
                    gt_ps = ps_t[:E, :128]
                    nc.tensor.transpose(gt_ps, g_sb[:, ti, :], id_sb[:, :])
                    nc.scalar.copy(gt_sb[:, ti * 128:(ti + 1) * 128], gt_ps)
                deferred.append(mk_t)
            for ti in range(TT):
                for pc in range(PC):
                    def mk_b(ti=ti, pc=pc):
                        ps_b = psum_aux.tile([128, PCHUNK], _F32,
                                             name=f"psb{ti}_{pc}", tag="px")
                        nc.tensor.matmul(
                            ps_b[:, :], gt_sb[:, ti * 128:(ti + 1) * 128],
                            eb_sb[:, pc * PCHUNK:(pc + 1) * PCHUNK],
                            start=True, stop=True)
                        acc_sl = acc[:, ti, pc * PCHUNK:(pc + 1) * PCHUNK]
                        nc.vector.tensor_add(acc_sl, acc_sl, ps_b[:, :])
                    deferred.append(mk_b)

        for e in range(1, E):
            if e == 1:
                wt = w1_pre + [w_dma(e, di) for di in range(len(w1_pre), DT)]
            else:
                wt = [w_dma(e, di) for di in range(DT)]
            for ti in range(TT):
                for pc in range(PC):
                    expert_group(e, wt, ti, pc)
                    if deferred:
                        deferred.pop(0)()

    nc.compile()
    return nc


def _get_module(mode: str, no_bias: bool = False) -> bass.Bass:
    key = (mode, no_bias)
    if key not in _build_cache:
        _build_cache[key] = _build(mode, no_bias)
    return _build_cache[key]


_last_results = None


def _host_inputs(x, gate_w, gate_b, expert_w, expert_b, mode,
                 no_bias=False):
    import ml_dtypes
    np_dt = ml_dtypes.bfloat16 if mode == "bf16" else np.float32

    x_flat = np.asarray(x, dtype=np.float32).reshape(TOK, D)
    gw_h = np.ascontiguousarray(np.asarray(gate_w, np.float32)).astype(np_dt)
    ew_h = np.ascontiguousarray(np.asarray(expert_w, np.float32)).astype(np_dt)
    base = {"gate_w": gw_h, "expert_w": ew_h}
    if not no_bias:
        gb_h = np.asarray(gate_b, np.float32).reshape(1, E)
        base["gb_rep"] = np.tile(gb_h, (1, TT)).astype(np_dt)
        base["expert_b"] = np.asarray(expert_b, np.float32).astype(
            ml_dtypes.bfloat16)
        base["ones"] = np.ones((1, 128), dtype=np_dt)
        base["ident"] = np.eye(128, dtype=np.float32)

    in_maps = []
    for c in range(N_CORES):
        shard = x_flat[c * TS:(c + 1) * TS]                  # [TS, D]
        xT_h = np.ascontiguousarray(shard.T).astype(np_dt)   # [D, TS]
        in_maps.append(dict(base, xT=xT_h))
    return in_maps


def kernel(x, gate_w, gate_b, expert_w, expert_b):
    global _last_results
    mode = MM_DTYPE
    no_bias = (not np.any(np.asarray(gate_b))
               and not np.any(np.asarray(expert_b)))
    nc = _get_module(mode, no_bias)
    in_maps = _host_inputs(x, gate_w, gate_b, expert_w, expert_b, mode,
                           no_bias)

    res = run_bass_kernel_spmd(nc, in_maps, core_ids=list(range(N_CORES)),
                               trace=TRACE)
    _last_results = res

    out = np.concatenate([res.results[c]["out"] for c in range(N_CORES)], axis=0)
    return out.reshape(B, T, P).astype(np.float32)
